# revision 21
# baseline (speedup 1.0000x reference)
"""Multi-head attention (N=4, L=2048, E=1024, H=16, DK=64) on 8 TRN2 cores.

The reference splits heads with a PLAIN RESHAPE (n, l, H*DK) -> (n, H, l, DK),
so "head" h is really a contiguous block of 128 tokens, and the 2048 attention
positions inside it are (token, s) pairs where s indexes sixteen 64-wide
E-slices.  Per (batch, block):
    Qb = q[n, 128b:128b+128, :].reshape(2048, 64)   (same for K, V)
    out_block = softmax(Qb Kb^T / 8) Vb  -> reshape(128, E) -> rows of out
Positions are processed in permuted order p' = 128*s + tok (a permutation of
the softmax axis; unpermuted on the way out).

Sharding: core c owns token rows [n, 256c : 256c+256) for every batch n (two
128-token blocks per batch).  Outputs are disjoint rows; the host scatters.
Each core gets the full weights (bf16, all resident in SBUF) and only its own
x columns.

v4 pipeline: the ScalarE exp stream (~1.15us per key tile, 256 tiles) is the
throughput floor; everything else is scheduled around keeping it saturated.
  - PE emission per key tile j: [scores(j); ~430ns of deferred work; PV(j-1)]
    so the exp latency of tile j-1 is hidden and per-tile PE time stays just
    under the exp time.
  - All projection / normalize / out-projection work is chopped into
    cost-classified pieces (heavy ~430ns PE, light DVE-only) in a
    deadline-keyed queue; forced pops guarantee producers are emitted before
    consumers, budget-based pacing keeps the PE load smooth.
  - DMA initiation order: x(batch0), Wk/Wq first columns, Wv first half --
    the minimal set for the first attention unit -- then the rest.
  - Softmax denominators: per-unit half-tile reciprocal_approx_fast so
    normalize/out-projection of the last unit is the only tail work.
"""

import bisect

import ml_dtypes
import numpy as np

import concourse.bass as bass
import concourse.mybir as mybir
import concourse.tile as tile
from concourse import bacc
from concourse.bass_utils import run_bass_kernel_spmd

N, L, E, H = 4, 2048, 1024, 16
DK = E // H  # 64
NC = 8
BPC = 2  # token blocks per core per batch
TPB = 128  # tokens per block
TPN = BPC * TPB  # 256 tokens per batch per core
TC = N * TPN  # 1024 tokens per core
P = 128
QC = 512  # q' chunk
NQC = 2048 // QC  # 4
NKT = 2048 // P  # 16 key tiles (= s values)
ET = E // P  # 8

F32 = mybir.dt.float32
BF16 = mybir.dt.bfloat16
MM_DT = BF16


def build_nc():
    nc = bacc.Bacc("TRN2", target_bir_lowering=False, debug=False, num_devices=NC)

    xTc = nc.dram_tensor("xTc", [E, TC], MM_DT, kind="ExternalInput").ap()
    wqT = nc.dram_tensor("wqT", [E, E], MM_DT, kind="ExternalInput").ap()
    wkT = nc.dram_tensor("wkT", [E, E], MM_DT, kind="ExternalInput").ap()
    wvT = nc.dram_tensor("wvT", [E, E], MM_DT, kind="ExternalInput").ap()
    woT = nc.dram_tensor("woT", [E, E], MM_DT, kind="ExternalInput").ap()
    outp = nc.dram_tensor("outp", [TC, E], F32, kind="ExternalOutput").ap()

    with tile.TileContext(nc) as tc:
        with (
            tc.tile_pool(name="const", bufs=1) as const,
            tc.tile_pool(name="wpool", bufs=1) as wpool,
            tc.tile_pool(name="xv", bufs=2) as xv_pool,
            tc.tile_pool(name="qk1", bufs=2) as qk1_pool,
            tc.tile_pool(name="expp", bufs=6) as exp_pool,
            tc.tile_pool(name="opt", bufs=2) as opt_pool,
            tc.tile_pool(name="nrm", bufs=2) as nrm_pool,
            tc.tile_pool(name="ops", bufs=2) as op_pool,
            tc.tile_pool(name="scps", bufs=2, space="PSUM") as sc_psum,
            tc.tile_pool(name="pvps", bufs=2, space="PSUM") as pv_psum,
            tc.tile_pool(name="fdps", bufs=2, space="PSUM") as fd_psum,
        ):
            ones_f32 = const.tile([P, P], F32)
            nc.vector.memset(ones_f32[:], 1.0)
            ones_r = const.tile([P, P], mybir.dt.float32r)
            nc.vector.tensor_copy(ones_r[:], ones_f32[:])

            # ---- input DMAs, initiated in consumer-priority order ----
            x_sb = xv_pool.tile([P, ET, TC], MM_DT, tag="xv", name="x_sb")
            xr_ = xTc.rearrange("(a p) t -> p a t", p=P)

            def w_tile(nm):
                return wpool.tile([P, ET, E], MM_DT, tag=nm, name=nm)

            wk_sb, wq_sb, wv_sb, wo_sb = (
                w_tile("wk"), w_tile("wq"), w_tile("wv"), w_tile("wo"))

            def wload(w_sb, w_dram, c0, c1):
                wr = w_dram.rearrange("(a p) d -> p a d", p=P)
                nc.sync.dma_start(
                    out=w_sb[:, :, c0 * P : c1 * P],
                    in_=wr[:, :, c0 * P : c1 * P],
                )

            nc.sync.dma_start(out=x_sb[:, :, 0:TPN], in_=xr_[:, :, 0:TPN])
            wload(wk_sb, wkT, 0, 4)
            wload(wq_sb, wqT, 0, 4)
            wload(wv_sb, wvT, 0, 4)
            wload(wk_sb, wkT, 4, 8)
            wload(wq_sb, wqT, 4, 8)
            wload(wv_sb, wvT, 4, 8)
            for n_ in range(1, N):
                nc.sync.dma_start(
                    out=x_sb[:, :, n_ * TPN : (n_ + 1) * TPN],
                    in_=xr_[:, :, n_ * TPN : (n_ + 1) * TPN],
                )
            wload(wo_sb, woT, 0, 8)

            def project_batch(n):
                """Pieces: (deadline_offset, cost, fn).  cost 1 = ~430ns of
                PE work, 0 = DVE-only.  scores(j) consume k-pair j//4 / q-pair
                u and are emitted BEFORE the drain of their slot (-2 margin);
                PV(j-1) consume v/ones after the drain."""
                v_sb = qk1_pool.tile(
                    [P, BPC, NKT, DK + 1], MM_DT, tag="v", name="v_sb"
                )
                q1t = qk1_pool.tile([P, 2048], MM_DT, tag="q1", name="q1t")
                k1t = qk1_pool.tile([P, 2048], MM_DT, tag="k1", name="k1t")

                def qk_pieces(w_sb, dst, pr_):
                    cell = {}

                    def mk_mm(sub, half):
                        def em():
                            if sub == 0 and half == 0:
                                cell["ps"] = fd_psum.tile(
                                    [P, 2, TPN], F32, tag="fd", name="qkps"
                                )
                            ps = cell["ps"]
                            a2 = 2 * pr_ + sub
                            for a in range(4 * half, 4 * half + 4):
                                nc.tensor.matmul(
                                    ps[:, sub, :],
                                    w_sb[:, a, a2 * P : (a2 + 1) * P],
                                    x_sb[:, a, n * TPN : (n + 1) * TPN],
                                    start=(a == 0),
                                    stop=(a == ET - 1),
                                )
                        return em

                    def evict():
                        ps = cell["ps"]
                        psr = ps.rearrange("q s (b t) -> q s b t", t=TPB)
                        dr = dst.rearrange("q (s t) -> q s t", t=TPB)
                        for rh in range(2):
                            for B in range(BPC):
                                s0 = 4 * pr_ + rh
                                nc.vector.tensor_copy(
                                    dr[B * DK : (B + 1) * DK, s0 : s0 + 3 : 2, :],
                                    psr[rh * DK : (rh + 1) * DK, :, B, :],
                                )

                    return [(1, mk_mm(s, h)) for s in range(2) for h in range(2)] \
                        + [(0, evict)]

                def v_pieces(B, eh):
                    cell = {}
                    tok0 = n * TPN + B * TPB

                    def mk_mm(q):
                        def em():
                            if q == 0:
                                cell["ps"] = fd_psum.tile(
                                    [P, 512], F32, tag="fd", name="vps"
                                )
                            ps = cell["ps"]
                            for a in range(2 * q, 2 * q + 2):
                                nc.tensor.matmul(
                                    ps[:],
                                    x_sb[:, a, tok0 : tok0 + TPB],
                                    wv_sb[:, a, eh * 512 : (eh + 1) * 512],
                                    start=(a == 0),
                                    stop=(a == ET - 1),
                                )
                        return em

                    def evict():
                        nc.vector.tensor_copy(
                            v_sb[:, B, eh * 8 : (eh + 1) * 8, 0:DK],
                            cell["ps"].rearrange("p (s d) -> p s d", d=DK),
                        )

                    return [(1, mk_mm(q)) for q in range(4)] + [(0, evict)]

                def ones_piece():
                    nc.vector.tensor_copy(
                        v_sb[:, :, :, DK], ones_f32[:, 0 : BPC * NKT]
                    )

                pieces = []

                def grp(off, lst):
                    pieces.extend((off, c, f) for c, f in lst)

                grp(-2, qk_pieces(wk_sb, k1t, 0))
                grp(-2, qk_pieces(wq_sb, q1t, 0))
                grp(0, v_pieces(0, 0))
                grp(0, v_pieces(1, 0))
                pieces.append((0, 0, ones_piece))
                grp(2, qk_pieces(wk_sb, k1t, 1))
                grp(6, qk_pieces(wk_sb, k1t, 2))
                grp(7, v_pieces(0, 1))
                grp(7, v_pieces(1, 1))
                grp(10, qk_pieces(wk_sb, k1t, 3))
                grp(14, qk_pieces(wq_sb, q1t, 1))
                grp(30, qk_pieces(wq_sb, q1t, 2))
                grp(46, qk_pieces(wq_sb, q1t, 3))
                return (v_sb, q1t, k1t), pieces

            def make_normalize_piece(opT, rec, B, u):
                def emit():
                    rp = 32 * (2 * (u % 2) + B)
                    bcp = fd_psum.tile([P, QC], F32, tag="fd", name="bcp")
                    nc.tensor.matmul(
                        bcp[:],
                        ones_r[rp : rp + 1, :],
                        rec[u // 2][rp : rp + 1, :],
                        start=True,
                        stop=True,
                        tile_position=(rp, 0),
                    )
                    for sg in range(2):
                        tgt = opT[sg * DK : (sg + 1) * DK,
                                  2 * u : 2 * u + 2, B, :]
                        nc.vector.tensor_mul(
                            tgt,
                            tgt,
                            bcp[sg * DK : (sg + 1) * DK, :].rearrange(
                                "d (sp t) -> d sp t", t=TPB
                            )[:, sg::2, :],
                        )
                return emit

            def make_outproj_pieces(opT, n, B, half):
                cell = {}
                r0 = n * TPN + B * TPB

                def mk_mm(q):
                    def em():
                        if q == 0:
                            cell["ps"] = fd_psum.tile([P, 512], F32,
                                                      tag="fd", name="opps")
                        ps = cell["ps"]
                        for a2 in range(2 * q, 2 * q + 2):
                            nc.tensor.matmul(
                                ps[:],
                                opT[:, a2, B, :],
                                wo_sb[:, a2, half * 512 : (half + 1) * 512],
                                start=(a2 == 0),
                                stop=(a2 == ET - 1),
                            )
                    return em

                def evict():
                    op_sb = op_pool.tile([P, 512], F32, tag="op")
                    nc.vector.tensor_copy(op_sb[:], cell["ps"][:])
                    nc.sync.dma_start(
                        out=outp[r0 : r0 + TPB, half * 512 : (half + 1) * 512],
                        in_=op_sb[:],
                    )

                return [(1, mk_mm(q)) for q in range(4)] + [(0, evict)]

            # ---- deadline-keyed deferred-work queue ----
            SPB = NQC * NKT + 2  # drain slots per batch (PV lags by 2)
            feed = []
            seq_counter = [0]

            def push(key, cost, piece, earliest=None):
                bisect.insort(
                    feed, (key, seq_counter[0], cost, earliest, piece))
                seq_counter[0] += 1

            def drain(slot, slots_left):
                spent = 0
                while feed and feed[0][0] <= slot:
                    it = feed.pop(0)
                    it[4]()
                    spent += it[2]
                popped = 0
                if feed:
                    want = (len(feed) + slots_left - 1) // max(slots_left, 1)
                    while feed and spent < 1 and popped < max(want, 1) + 1:
                        if feed[0][3] is not None and feed[0][3] > slot:
                            break  # inputs still in flight; don't stall PE
                        it = feed.pop(0)
                        it[4]()
                        spent += it[2]
                        popped += 1

            tiles, pieces0 = project_batch(0)
            for off, cost, p in pieces0:
                if off < 0:
                    p()  # batch 0's k0/q0 run eagerly
                else:
                    push(off, cost, p)
            next_state = None

            for n in range(N):
                base = n * SPB
                v_sb, q1t, k1t = tiles
                if n + 1 < N:
                    next_state = project_batch(n + 1)
                    for off, cost, p in next_state[1]:
                        push((n + 1) * SPB + off, cost, p)
                opT = opt_pool.tile([P, ET, BPC, TPB], MM_DT, tag="opT",
                                    name="opT")
                # denominators: tile u//2, partition half u%2, row 32*(2*(u%2)+B)
                sums = [
                    nrm_pool.tile([P, QC], F32, tag="sums", name=f"sums{_i}")
                    for _i in range(2)
                ]
                # only 4 partitions per tile carry data; define the rest so
                # the half-tile reciprocals read initialized memory
                for _i in range(2):
                    nc.vector.memset(sums[_i][:], 1.0)
                rec = [
                    nrm_pool.tile([P, QC], mybir.dt.float32r, tag="rec",
                                  name=f"rec{_i}")
                    for _i in range(2)
                ]
                recf = nrm_pool.tile([P, QC], F32, tag="recf", name="recf")

                pv_of = {}

                def evict_unit(pu):
                    pv = pv_of.pop(pu)
                    hf = pu % 2
                    for B in range(BPC):
                        # unnormalized eviction into opT; s = 4*pu + sp
                        for sg in range(2):
                            nc.vector.tensor_copy(
                                opT[sg * DK : (sg + 1) * DK,
                                    2 * pu : 2 * pu + 2, B, :],
                                pv[B][0:DK, :].rearrange(
                                    "d (sp t) -> d sp t", t=TPB
                                )[:, sg::2, :],
                            )
                        rp = 32 * (2 * hf + B)
                        nc.vector.tensor_copy(
                            sums[pu // 2][rp : rp + 1, :],
                            pv[B][DK : DK + 1, :],
                        )
                    # full-tile reciprocal (the custom DVE op requires
                    # partition offset 0 on HW); unwritten rows are memset
                    # so they are defined.  Copy just this unit's half.
                    psl = slice(64 * hf, 64 * hf + 64)
                    with nc.allow_low_precision(reason="softmax denominators"):
                        nc.vector.reciprocal_approx_fast(
                            out=recf[:], in_=sums[pu // 2][:]
                        )
                        nc.vector.tensor_copy(rec[pu // 2][psl, :],
                                              recf[psl, :])
                    # earliest: give the DVE time to finish the rec copy
                    # before the PE-side broadcast can be popped
                    esl = base + pu * NKT + 23
                    nkey = (base + (pu + 1) * NKT + 14 if pu < NQC - 1
                            else base + SPB + 8)
                    for B in range(BPC):
                        push(nkey, 1, make_normalize_piece(opT, rec, B, pu),
                             earliest=esl)

                # flat slot stream over (u, j): scores/exp lead PV by TWO
                # slots across unit boundaries -- by PV time its exp has long
                # retired, so the PE never breaks its p-state streak on an
                # exp semaphore
                exps_hist = {}
                for k in range(NQC * NKT + 2):
                    exps_new = None
                    if k < NQC * NKT:
                        u, j = divmod(k, NKT)
                        sc = sc_psum.tile([P, BPC, QC], F32, tag="sc")
                        ksl = slice(j * TPB, (j + 1) * TPB)
                        qsl = slice(u * QC, (u + 1) * QC)
                        for B in range(BPC):
                            bsl = slice(B * DK, (B + 1) * DK)
                            nc.tensor.matmul(
                                sc[:, B, :],
                                k1t[bsl, ksl],
                                q1t[bsl, qsl],
                                start=True,
                                stop=True,
                            )
                        exps_new = exp_pool.tile([P, BPC, QC], MM_DT,
                                                 tag="exps")
                        nc.scalar.activation(
                            exps_new[:],
                            sc[:],
                            mybir.ActivationFunctionType.Exp,
                            scale=1.0 / np.sqrt(DK),
                        )
                    if exps_new is not None:
                        exps_hist[k] = exps_new
                    drain(base + k, NQC * NKT + 2 - k)
                    if k >= 2:
                        pu, pj = divmod(k - 2, NKT)
                        if pj == 0:
                            pv_of[pu] = [
                                pv_psum.tile([DK + 1, QC], F32, tag="pv",
                                             name=f"pv{_b}")
                                for _b in range(BPC)
                            ]
                        exps_p = exps_hist.pop(k - 2)
                        for B in range(BPC):
                            nc.tensor.matmul(
                                pv_of[pu][B][:],
                                v_sb[:, B, pj, :],
                                exps_p[:, B, :],
                                start=(pj == 0),
                                stop=(pj == NKT - 1),
                            )
                        if pj == NKT - 1:
                            evict_unit(pu)
                for B in range(BPC):
                    for half in range(2):
                        for cost, p in make_outproj_pieces(opT, n, B, half):
                            push(base + SPB + 20, cost, p,
                                 earliest=base + SPB + 16)
                if next_state is not None:
                    tiles = next_state[0]

            while feed:
                feed.pop(0)[4]()

    nc.compile()
    return nc


_CACHED_NC = None


def get_nc():
    global _CACHED_NC
    if _CACHED_NC is None:
        _CACHED_NC = build_nc()
    return _CACHED_NC


def make_in_maps(inputs):
    x = np.ascontiguousarray(np.asarray(inputs["x"], dtype=np.float32))
    Wq = np.asarray(inputs["Wq"], dtype=np.float32)
    Wk = np.asarray(inputs["Wk"], dtype=np.float32)
    Wv = np.asarray(inputs["Wv"], dtype=np.float32)
    Wo = np.asarray(inputs["Wo"], dtype=np.float32)

    def cast(a):
        return np.ascontiguousarray(a).astype(ml_dtypes.bfloat16)

    wqT = cast(Wq.T)
    wkT = cast(Wk.T)
    wvT = cast(Wv.T)
    woT = cast(Wo.T)
    xr = x.reshape(N, L, E)

    in_maps = []
    for c in range(NC):
        xc = np.concatenate(
            [xr[n, 256 * c : 256 * (c + 1), :] for n in range(N)], axis=0
        )
        in_maps.append(
            {
                "xTc": cast(xc.T),
                "wqT": wqT,
                "wkT": wkT,
                "wvT": wvT,
                "woT": woT,
            }
        )
    return in_maps


def kernel(x, Wq, Wk, Wv, Wo):
    in_maps = make_in_maps({"x": x, "Wq": Wq, "Wk": Wk, "Wv": Wv, "Wo": Wo})
    res = run_bass_kernel_spmd(get_nc(), in_maps, list(range(NC)))
    out = np.empty((N, L, E), dtype=np.float32)
    for c in range(NC):
        o = res.results[c]["outp"].reshape(N, TPN, E)
        out[:, 256 * c : 256 * (c + 1), :] = o
    return out


# revision 25
# speedup vs baseline: 1.1885x; 1.1885x over previous
"""Multi-head attention (N=4, L=2048, E=1024, H=16, DK=64) on 8 TRN2 cores.

The reference splits heads with a PLAIN RESHAPE (n, l, H*DK) -> (n, H, l, DK),
so "head" h is really a contiguous block of 128 tokens, and the 2048 attention
positions inside it are (token, s) pairs where s indexes sixteen 64-wide
E-slices.  Per (batch, block):
    Qb = q[n, 128b:128b+128, :].reshape(2048, 64)   (same for K, V)
    out_block = softmax(Qb Kb^T / 8) Vb  -> reshape(128, E) -> rows of out
Positions are processed in permuted order p' = 128*s + tok (a permutation of
the softmax axis; unpermuted on the way out).

Sharding: core c owns token rows [n, 256c : 256c+256) for every batch n (two
128-token blocks per batch).  Outputs are disjoint rows; the host scatters.
Each core gets the full weights (bf16, all resident in SBUF) and only its own
x columns.

v4 pipeline: the ScalarE exp stream (~1.15us per key tile, 256 tiles) is the
throughput floor; everything else is scheduled around keeping it saturated.
  - PE emission per key tile j: [scores(j); ~430ns of deferred work; PV(j-1)]
    so the exp latency of tile j-1 is hidden and per-tile PE time stays just
    under the exp time.
  - All projection / normalize / out-projection work is chopped into
    cost-classified pieces (heavy ~430ns PE, light DVE-only) in a
    deadline-keyed queue; forced pops guarantee producers are emitted before
    consumers, budget-based pacing keeps the PE load smooth.
  - DMA initiation order: x(batch0), Wk/Wq first columns, Wv first half --
    the minimal set for the first attention unit -- then the rest.
  - Softmax denominators: per-unit half-tile reciprocal_approx_fast so
    normalize/out-projection of the last unit is the only tail work.
"""

import bisect

import ml_dtypes
import numpy as np

import concourse.bass as bass
import concourse.mybir as mybir
import concourse.tile as tile
from concourse import bacc
from concourse.bass_utils import run_bass_kernel_spmd

N, L, E, H = 4, 2048, 1024, 16
DK = E // H  # 64
NC = 8
BPC = 2  # token blocks per core per batch
TPB = 128  # tokens per block
TPN = BPC * TPB  # 256 tokens per batch per core
TC = N * TPN  # 1024 tokens per core
P = 128
QC = 512  # q' chunk
NQC = 2048 // QC  # 4
NKT = 2048 // P  # 16 key tiles (= s values)
ET = E // P  # 8

F32 = mybir.dt.float32
BF16 = mybir.dt.bfloat16
MM_DT = BF16


def build_nc():
    nc = bacc.Bacc("TRN2", target_bir_lowering=False, debug=False, num_devices=NC)

    xTc = nc.dram_tensor("xTc", [E, TC], MM_DT, kind="ExternalInput").ap()
    wqT = nc.dram_tensor("wqT", [E, E], MM_DT, kind="ExternalInput").ap()
    wkT = nc.dram_tensor("wkT", [E, E], MM_DT, kind="ExternalInput").ap()
    wvT = nc.dram_tensor("wvT", [E, E], MM_DT, kind="ExternalInput").ap()
    woT = nc.dram_tensor("woT", [E, E], MM_DT, kind="ExternalInput").ap()
    outp = nc.dram_tensor("outp", [TC, E], F32, kind="ExternalOutput").ap()

    with tile.TileContext(nc) as tc:
        with (
            tc.tile_pool(name="const", bufs=1) as const,
            tc.tile_pool(name="wpool", bufs=1) as wpool,
            tc.tile_pool(name="xv", bufs=2) as xv_pool,
            tc.tile_pool(name="qk1", bufs=2) as qk1_pool,
            tc.tile_pool(name="expp", bufs=6) as exp_pool,
            tc.tile_pool(name="opt", bufs=2) as opt_pool,
            tc.tile_pool(name="nrm", bufs=2) as nrm_pool,
            tc.tile_pool(name="ops", bufs=2) as op_pool,
            tc.tile_pool(name="scps", bufs=2, space="PSUM") as sc_psum,
            tc.tile_pool(name="pvps", bufs=2, space="PSUM") as pv_psum,
            tc.tile_pool(name="fdps", bufs=2, space="PSUM") as fd_psum,
        ):
            ones_f32 = const.tile([P, P], F32)
            nc.vector.memset(ones_f32[:], 1.0)
            ones_r = const.tile([P, P], mybir.dt.float32r)
            nc.vector.tensor_copy(ones_r[:], ones_f32[:])

            # ---- input DMAs, initiated in consumer-priority order ----
            x_sb = xv_pool.tile([P, ET, TC], MM_DT, tag="xv", name="x_sb")
            xr_ = xTc.rearrange("(a p) t -> p a t", p=P)

            def w_tile(nm):
                return wpool.tile([P, ET, E], MM_DT, tag=nm, name=nm)

            wk_sb, wq_sb, wv_sb, wo_sb = (
                w_tile("wk"), w_tile("wq"), w_tile("wv"), w_tile("wo"))

            def wload(w_sb, w_dram, c0, c1):
                wr = w_dram.rearrange("(a p) d -> p a d", p=P)
                nc.sync.dma_start(
                    out=w_sb[:, :, c0 * P : c1 * P],
                    in_=wr[:, :, c0 * P : c1 * P],
                )

            nc.sync.dma_start(out=x_sb[:, :, 0:TPN], in_=xr_[:, :, 0:TPN])
            wload(wk_sb, wkT, 0, 4)
            wload(wq_sb, wqT, 0, 4)
            wload(wv_sb, wvT, 0, 4)
            wload(wk_sb, wkT, 4, 8)
            wload(wq_sb, wqT, 4, 8)
            wload(wv_sb, wvT, 4, 8)
            for n_ in range(1, N):
                nc.sync.dma_start(
                    out=x_sb[:, :, n_ * TPN : (n_ + 1) * TPN],
                    in_=xr_[:, :, n_ * TPN : (n_ + 1) * TPN],
                )
            wload(wo_sb, woT, 0, 8)

            def project_batch(n):
                """Pieces: (deadline_offset, cost, fn).  cost 1 = ~430ns of
                PE work, 0 = DVE-only.  scores(j) consume k-pair j//4 / q-pair
                u and are emitted BEFORE the drain of their slot (-2 margin);
                PV(j-1) consume v/ones after the drain."""
                v_sb = qk1_pool.tile(
                    [P, BPC, NKT, DK + 1], MM_DT, tag="v", name="v_sb"
                )
                q1t = qk1_pool.tile([P, 2048], MM_DT, tag="q1", name="q1t")
                k1t = qk1_pool.tile([P, 2048], MM_DT, tag="k1", name="k1t")

                def qk_pieces(w_sb, dst, pr_):
                    cell = {}

                    def mk_mm(sub, half):
                        def em():
                            if sub == 0 and half == 0:
                                cell["ps"] = fd_psum.tile(
                                    [P, 2, TPN], F32, tag="fd", name="qkps"
                                )
                            ps = cell["ps"]
                            a2 = 2 * pr_ + sub
                            for a in range(4 * half, 4 * half + 4):
                                nc.tensor.matmul(
                                    ps[:, sub, :],
                                    w_sb[:, a, a2 * P : (a2 + 1) * P],
                                    x_sb[:, a, n * TPN : (n + 1) * TPN],
                                    start=(a == 0),
                                    stop=(a == ET - 1),
                                )
                        return em

                    def evict():
                        ps = cell["ps"]
                        psr = ps.rearrange("q s (b t) -> q s b t", t=TPB)
                        dr = dst.rearrange("q (s t) -> q s t", t=TPB)
                        for rh in range(2):
                            for B in range(BPC):
                                s0 = 4 * pr_ + rh
                                nc.vector.tensor_copy(
                                    dr[B * DK : (B + 1) * DK, s0 : s0 + 3 : 2, :],
                                    psr[rh * DK : (rh + 1) * DK, :, B, :],
                                )

                    return [(1, mk_mm(s, h)) for s in range(2) for h in range(2)] \
                        + [(0, evict)]

                def v_pieces(B, eh):
                    cell = {}
                    tok0 = n * TPN + B * TPB

                    def mk_mm(q):
                        def em():
                            if q == 0:
                                cell["ps"] = fd_psum.tile(
                                    [P, 512], F32, tag="fd", name="vps"
                                )
                            ps = cell["ps"]
                            for a in range(2 * q, 2 * q + 2):
                                nc.tensor.matmul(
                                    ps[:],
                                    x_sb[:, a, tok0 : tok0 + TPB],
                                    wv_sb[:, a, eh * 512 : (eh + 1) * 512],
                                    start=(a == 0),
                                    stop=(a == ET - 1),
                                )
                        return em

                    def evict():
                        nc.vector.tensor_copy(
                            v_sb[:, B, eh * 8 : (eh + 1) * 8, 0:DK],
                            cell["ps"].rearrange("p (s d) -> p s d", d=DK),
                        )

                    return [(1, mk_mm(q)) for q in range(4)] + [(0, evict)]

                def ones_piece():
                    nc.vector.tensor_copy(
                        v_sb[:, :, :, DK], ones_f32[:, 0 : BPC * NKT]
                    )

                pieces = []

                def grp(off, lst):
                    pieces.extend((off, c, f) for c, f in lst)

                grp(-2, qk_pieces(wk_sb, k1t, 0))
                grp(-2, qk_pieces(wq_sb, q1t, 0))
                grp(0, v_pieces(0, 0))
                grp(0, v_pieces(1, 0))
                pieces.append((0, 0, ones_piece))
                grp(2, qk_pieces(wk_sb, k1t, 1))
                grp(6, qk_pieces(wk_sb, k1t, 2))
                grp(7, v_pieces(0, 1))
                grp(7, v_pieces(1, 1))
                grp(10, qk_pieces(wk_sb, k1t, 3))
                grp(14, qk_pieces(wq_sb, q1t, 1))
                grp(30, qk_pieces(wq_sb, q1t, 2))
                grp(46, qk_pieces(wq_sb, q1t, 3))
                return (v_sb, q1t, k1t), pieces

            def make_normalize_piece(opT, rec, B, u):
                def emit():
                    rp = 32 * (2 * (u % 2) + B)
                    bcp = fd_psum.tile([P, QC], F32, tag="fd", name="bcp")
                    nc.tensor.matmul(
                        bcp[:],
                        ones_r[rp : rp + 1, :],
                        rec[u // 2][rp : rp + 1, :],
                        start=True,
                        stop=True,
                        tile_position=(rp, 0),
                    )
                    for sg in range(2):
                        tgt = opT[sg * DK : (sg + 1) * DK,
                                  2 * u : 2 * u + 2, B, :]
                        nc.vector.tensor_mul(
                            tgt,
                            tgt,
                            bcp[sg * DK : (sg + 1) * DK, :].rearrange(
                                "d (sp t) -> d sp t", t=TPB
                            )[:, sg::2, :],
                        )
                return emit

            def make_outproj_pieces(opT, n, B, half):
                cell = {}
                r0 = n * TPN + B * TPB

                def mk_mm(q):
                    def em():
                        if q == 0:
                            cell["ps"] = fd_psum.tile([P, 512], F32,
                                                      tag="fd", name="opps")
                        ps = cell["ps"]
                        for a2 in range(2 * q, 2 * q + 2):
                            nc.tensor.matmul(
                                ps[:],
                                opT[:, a2, B, :],
                                wo_sb[:, a2, half * 512 : (half + 1) * 512],
                                start=(a2 == 0),
                                stop=(a2 == ET - 1),
                            )
                    return em

                def evict():
                    op_sb = op_pool.tile([P, 512], F32, tag="op")
                    nc.vector.tensor_copy(op_sb[:], cell["ps"][:])
                    nc.sync.dma_start(
                        out=outp[r0 : r0 + TPB, half * 512 : (half + 1) * 512],
                        in_=op_sb[:],
                    )

                return [(1, mk_mm(q)) for q in range(4)] + [(0, evict)]

            # ---- deadline-keyed deferred-work queue ----
            SPB = NQC * NKT + 1  # drain slots per batch
            feed = []
            seq_counter = [0]

            def push(key, cost, piece, earliest=None):
                bisect.insort(
                    feed, (key, seq_counter[0], cost, earliest, piece))
                seq_counter[0] += 1

            def drain(slot, slots_left):
                spent = 0
                while feed and feed[0][0] <= slot:
                    it = feed.pop(0)
                    it[4]()
                    spent += it[2]
                popped = 0
                if feed:
                    want = (len(feed) + slots_left - 1) // max(slots_left, 1)
                    while feed and spent < 1 and popped < max(want, 1) + 1:
                        if feed[0][3] is not None and feed[0][3] > slot:
                            break  # inputs still in flight; don't stall PE
                        it = feed.pop(0)
                        it[4]()
                        spent += it[2]
                        popped += 1

            tiles, pieces0 = project_batch(0)
            for off, cost, p in pieces0:
                if off < 0:
                    p()  # batch 0's k0/q0 run eagerly
                else:
                    push(off, cost, p)
            next_state = None

            for n in range(N):
                base = n * SPB
                v_sb, q1t, k1t = tiles
                if n + 1 < N:
                    next_state = project_batch(n + 1)
                    for off, cost, p in next_state[1]:
                        push((n + 1) * SPB + off, cost, p)
                opT = opt_pool.tile([P, ET, BPC, TPB], MM_DT, tag="opT",
                                    name="opT")
                # denominators: tile u//2, partition half u%2, row 32*(2*(u%2)+B)
                sums = [
                    nrm_pool.tile([P, QC], F32, tag="sums", name=f"sums{_i}")
                    for _i in range(2)
                ]
                # only 4 partitions per tile carry data; define the rest so
                # the half-tile reciprocals read initialized memory
                for _i in range(2):
                    nc.vector.memset(sums[_i][:], 1.0)
                rec = [
                    nrm_pool.tile([P, QC], mybir.dt.float32r, tag="rec",
                                  name=f"rec{_i}")
                    for _i in range(2)
                ]
                recf = nrm_pool.tile([P, QC], F32, tag="recf", name="recf")

                pv_of = {}

                def evict_unit(pu):
                    pv = pv_of.pop(pu)
                    hf = pu % 2
                    for B in range(BPC):
                        # unnormalized eviction into opT; s = 4*pu + sp
                        for sg in range(2):
                            nc.vector.tensor_copy(
                                opT[sg * DK : (sg + 1) * DK,
                                    2 * pu : 2 * pu + 2, B, :],
                                pv[B][0:DK, :].rearrange(
                                    "d (sp t) -> d sp t", t=TPB
                                )[:, sg::2, :],
                            )
                        rp = 32 * (2 * hf + B)
                        nc.vector.tensor_copy(
                            sums[pu // 2][rp : rp + 1, :],
                            pv[B][DK : DK + 1, :],
                        )
                    # full-tile reciprocal (the custom DVE op requires
                    # partition offset 0 on HW); unwritten rows are memset
                    # so they are defined.  Copy just this unit's half.
                    psl = slice(64 * hf, 64 * hf + 64)
                    with nc.allow_low_precision(reason="softmax denominators"):
                        nc.vector.reciprocal_approx_fast(
                            out=recf[:], in_=sums[pu // 2][:]
                        )
                        nc.vector.tensor_copy(rec[pu // 2][psl, :],
                                              recf[psl, :])
                    # earliest: give the DVE time to finish the rec copy
                    # before the PE-side broadcast can be popped
                    esl = base + pu * NKT + 21
                    for B in range(BPC):
                        if pu < NQC - 1:
                            nkey = base + (pu + 1) * NKT + 14
                        else:
                            # last unit: on the final batch interleave with
                            # the out-projections (norm B0, outproj B0, ...)
                            nkey = base + SPB + 8 + 2 * B
                        push(nkey, 1, make_normalize_piece(opT, rec, B, pu),
                             earliest=esl)

                # flat slot stream over (u, j): scores/exp lead PV by one
                # slot across unit boundaries.  (A 2-slot lead was tried and
                # made every engine ~20% slower -- the denser overlap drops
                # the core DVFS clock -- so lag-1 is the sweet spot.)
                exps_hist = {}
                for k in range(NQC * NKT + 1):
                    exps_new = None
                    if k < NQC * NKT:
                        u, j = divmod(k, NKT)
                        sc = sc_psum.tile([P, BPC, QC], F32, tag="sc")
                        ksl = slice(j * TPB, (j + 1) * TPB)
                        qsl = slice(u * QC, (u + 1) * QC)
                        for B in range(BPC):
                            bsl = slice(B * DK, (B + 1) * DK)
                            nc.tensor.matmul(
                                sc[:, B, :],
                                k1t[bsl, ksl],
                                q1t[bsl, qsl],
                                start=True,
                                stop=True,
                            )
                        exps_new = exp_pool.tile([P, BPC, QC], MM_DT,
                                                 tag="exps")
                        nc.scalar.activation(
                            exps_new[:],
                            sc[:],
                            mybir.ActivationFunctionType.Exp,
                            scale=1.0 / np.sqrt(DK),
                        )
                    if exps_new is not None:
                        exps_hist[k] = exps_new
                    drain(base + k, NQC * NKT + 1 - k)
                    if k >= 1:
                        pu, pj = divmod(k - 1, NKT)
                        if pj == 0:
                            pv_of[pu] = [
                                pv_psum.tile([DK + 1, QC], F32, tag="pv",
                                             name=f"pv{_b}")
                                for _b in range(BPC)
                            ]
                        exps_p = exps_hist.pop(k - 1)
                        for B in range(BPC):
                            nc.tensor.matmul(
                                pv_of[pu][B][:],
                                v_sb[:, B, pj, :],
                                exps_p[:, B, :],
                                start=(pj == 0),
                                stop=(pj == NKT - 1),
                            )
                        if pj == NKT - 1:
                            evict_unit(pu)
                for B in range(BPC):
                    okey = base + SPB + (9 + 2 * B if n == N - 1 else 20)
                    for half in range(2):
                        for cost, p in make_outproj_pieces(opT, n, B, half):
                            push(okey, cost, p, earliest=base + SPB + 12)
                if next_state is not None:
                    tiles = next_state[0]

            while feed:
                feed.pop(0)[4]()

    nc.compile()
    return nc


_CACHED_NC = None


def get_nc():
    global _CACHED_NC
    if _CACHED_NC is None:
        _CACHED_NC = build_nc()
    return _CACHED_NC


def make_in_maps(inputs):
    x = np.ascontiguousarray(np.asarray(inputs["x"], dtype=np.float32))
    Wq = np.asarray(inputs["Wq"], dtype=np.float32)
    Wk = np.asarray(inputs["Wk"], dtype=np.float32)
    Wv = np.asarray(inputs["Wv"], dtype=np.float32)
    Wo = np.asarray(inputs["Wo"], dtype=np.float32)

    def cast(a):
        return np.ascontiguousarray(a).astype(ml_dtypes.bfloat16)

    wqT = cast(Wq.T)
    wkT = cast(Wk.T)
    wvT = cast(Wv.T)
    woT = cast(Wo.T)
    xr = x.reshape(N, L, E)

    in_maps = []
    for c in range(NC):
        xc = np.concatenate(
            [xr[n, 256 * c : 256 * (c + 1), :] for n in range(N)], axis=0
        )
        in_maps.append(
            {
                "xTc": cast(xc.T),
                "wqT": wqT,
                "wkT": wkT,
                "wvT": wvT,
                "woT": woT,
            }
        )
    return in_maps


def kernel(x, Wq, Wk, Wv, Wo):
    in_maps = make_in_maps({"x": x, "Wq": Wq, "Wk": Wk, "Wv": Wv, "Wo": Wo})
    res = run_bass_kernel_spmd(get_nc(), in_maps, list(range(NC)))
    out = np.empty((N, L, E), dtype=np.float32)
    for c in range(NC):
        o = res.results[c]["outp"].reshape(N, TPN, E)
        out[:, 256 * c : 256 * (c + 1), :] = o
    return out
